# revision 15
# baseline (speedup 1.0000x reference)
"""Trainium2 Bass kernel for nn_Attention_46780783788294.

Multi-head causal-ish attention (mask fills with 0.0, not -inf) for
x:[2,2048,1024], 16 heads of d_head=64, fp32 in/out, bf16 compute.

Sharding: 8 cores = 2 batches x 4 head-groups (4 heads each). Each core
computes its batch/head-group partial output [2048,1024] (bf16); host sums
the 4 partials per batch in fp32 and adds b_O.

v2 design (all-transposed "S^T" layout, bf16 matmuls, fp32 PSUM):
  - Scores per head pair are row-tiled on the PE (two 64-contraction
    matmuls in PE row halves run concurrently via auto tile_position).
  - AV is col-tiled: the two heads' [128k x 64d] V matmuls run
    concurrently in PE column halves (out partitions 0:64 / 64:128 of one
    zps bank).  No ones-column in V; instead the softmax denominator is
    computed from PTS = sum_j exp-tile (DVE bf16 accumulation) with a tiny
    ones-matmul per (chunk, pair), plus an analytic masked-count constant
    (masked entries contribute exp(0)=1 each).
  - Skipped fully-masked k-blocks contribute their V column-sums
    analytically (suffix sfx / per-block b_sb correction matmuls, also
    col-tiled per head half).
  - PE warmup burst at t~1us flips the HAM clock gate to 2.4 GHz before
    the first real matmul.
  - Intro DMAs are coalesced with 3D access patterns and spread across
    sync/scalar/gpsimd dispatchers for a fast ramp.
  - Emission weaves "filler" matmul work (projections of the next chunk,
    output projection of the previous chunk, V/b_sb/sfx) between attention
    blocks so the PE never idles while the ACT engine runs exp.
"""

import os
import sys

import numpy as np


def _ensure_concourse():
    try:
        import concourse  # noqa: F401
    except ImportError:
        for p in ("/root/.axon_site", "/root/.axon_site/_ro/trn_rl_repo",
                  "/root/.axon_site/_ro/pypackages", "/opt/trn_rl_repo"):
            if os.path.isdir(p) and p not in sys.path:
                sys.path.append(p)


_ensure_concourse()

import concourse.bass as bass  # noqa: E402
import concourse.tile as tile  # noqa: E402
from concourse import bacc, mybir  # noqa: E402
from concourse import bass_utils  # noqa: E402
from contextlib import ExitStack  # noqa: E402
from collections import deque  # noqa: E402

F32 = mybir.dt.float32
BF16 = mybir.dt.bfloat16
EXP = mybir.ActivationFunctionType.Exp

S = 2048      # sequence length
M = 1024      # d_model
DH = 64       # d_head
HL = 4        # heads per core
NP = 2        # head pairs per core
CH = 512      # q-chunk width
NCH = S // CH     # 4 q chunks
KB = S // 128     # 16 k blocks
MB = M // 128     # 8 m blocks
N_CORES = 8
NWARM = 36


def _emit(tc, nc, d, zero_bias):
    mm = nc.tensor.matmul
    with ExitStack() as ctx:
        # ---- persistent pools ----
        qkp = ctx.enter_context(tc.tile_pool(name="qkp", bufs=1))
        vp = ctx.enter_context(tc.tile_pool(name="vp", bufs=1))
        wop = ctx.enter_context(tc.tile_pool(name="wop", bufs=1))
        cst = ctx.enter_context(tc.tile_pool(name="cst", bufs=1))
        dnp = ctx.enter_context(tc.tile_pool(name="dnp", bufs=1))
        z2p = ctx.enter_context(tc.tile_pool(name="z2p", bufs=1))
        xp = ctx.enter_context(tc.tile_pool(name="xp", bufs=1))
        wp = ctx.enter_context(tc.tile_pool(name="wp", bufs=1))
        pp = ctx.enter_context(tc.tile_pool(name="pp", bufs=4))
        stg = ctx.enter_context(tc.tile_pool(name="stg", bufs=4))
        op_sb = ctx.enter_context(tc.tile_pool(name="op_sb", bufs=4))
        # PSUM: 8 banks total.  psS = scores [128,1024] (2 banks) x2 bufs,
        # psZ = z accum [128,512] x2 pair tags, psX = rotating 1-bank pool
        # for everything else (qk/v/outproj/bc/D/warmup).
        psX = ctx.enter_context(tc.tile_pool(name="psX", bufs=2, space="PSUM"))
        psS = ctx.enter_context(tc.tile_pool(name="psS", bufs=2, space="PSUM"))
        psZ = ctx.enter_context(tc.tile_pool(name="psZ", bufs=1, space="PSUM"))

        qt = [qkp.tile([128, S], BF16, name=f"qt{p}") for p in range(NP)]
        kt = [qkp.tile([128, S], BF16, name=f"kt{p}") for p in range(NP)]
        vt = [vp.tile([128, 256], BF16, name=f"vt{j}") for j in range(KB)]
        wo_t = [wop.tile([128, M], BF16, name=f"wo{p}") for p in range(NP)]
        dtri = cst.tile([128, 256], BF16, name="dtri")
        e0t = cst.tile([1, 128], BF16, name="e0")
        e1t = cst.tile([1, 128], BF16, name="e1")
        ones_row = cst.tile([1, CH], BF16, name="ones_row")
        ones_col = cst.tile([128, 1], BF16, name="ones_col")
        wu = cst.tile([128, 128], BF16, name="wu")
        dk_t = cst.tile([1, S], F32, name="dk")
        z2u = [z2p.tile([128, S], BF16, name=f"z2u{p}") for p in range(NP)]
        pts = [dnp.tile([128, 2 * CH], BF16, name=f"pts{p}") for p in range(NP)]
        sfx = [dnp.tile([1, 256], BF16, name=f"sfx{c}") for c in range(3)]
        b_sb = {j: dnp.tile([1, 256], BF16, name=f"bsb{j}") for j in range(KB)
                if j % 4}

        xt_all = xp.tile([128, MB * S], BF16, name="xt")
        wq_all = wp.tile([128, MB * 256], BF16, name="wq")
        wk_all = wp.tile([128, MB * 256], BF16, name="wk")
        wv_all = wp.tile([128, MB * 256], BF16, name="wv")
        if not zero_bias:
            xt_ones = xp.tile([1, S], BF16, name="xt_ones")
            wq_b = wp.tile([1, 256], BF16, name="wq_b")
            wk_b = wp.tile([1, 256], BF16, name="wk_b")
            wv_b = wp.tile([1, 256], BF16, name="wv_b")

        # ---- warmup: get the PE HAM clock gate to 8/8 before real work ----
        nc.gpsimd.memset(wu[:], 0.01)
        wups = psX.tile([128, 128], F32, name="wups", tag="px")
        for _ in range(NWARM):
            mm(wups[:], wu[:], wu[:], start=True, stop=True)

        # ---- intro DMAs: coalesced 3D APs, spread over 3 dispatchers ----
        def w3d(dst, src_t, lo, hi):
            nmb = hi - lo
            dv = dst.rearrange("p (m c) -> p m c", m=MB)[:, lo:hi, :]
            sv = src_t[128 * lo:128 * hi, :].rearrange("(m p) c -> p m c",
                                                       m=nmb)
            return dv, sv

        # first-matmul gates: wq (sync) and xt chunk 0 (scalar), in halves
        for lo, hi in ((0, 4), (4, 8)):
            dv, sv = w3d(wq_all, d["wq"], lo, hi)
            nc.sync.dma_start(dv, sv)
            dxv = xt_all.rearrange("p (m c) -> p m c", m=MB)[:, lo:hi, 0:CH]
            sxv = d["xt"][128 * lo:128 * hi, 0:CH].rearrange(
                "(m p) c -> p m c", m=hi - lo)
            nc.scalar.dma_start(dxv, sxv)
        for lo, hi in ((0, 4), (4, 8)):
            dv, sv = w3d(wk_all, d["wk"], lo, hi)
            nc.sync.dma_start(dv, sv)
            dv, sv = w3d(wv_all, d["wv"], lo, hi)
            nc.scalar.dma_start(dv, sv)
        nc.scalar.dma_start(dtri[:], d["mk"][:])
        nc.sync.dma_start(ones_row[:], d["cst"][0:1, :])
        nc.sync.dma_start(ones_col[:], d["cst"][0:128, 0:1])
        nc.sync.dma_start(e0t[:], d["e2"][0:1, :])
        nc.sync.dma_start(e1t[:], d["e2"][1:2, :])
        nc.sync.dma_start(dk_t[:], d["dk"][:])
        if not zero_bias:
            nc.gpsimd.dma_start(wq_b[:], d["wq"][1024:1025, :])
            nc.gpsimd.dma_start(wk_b[:], d["wk"][1024:1025, :])
            nc.gpsimd.dma_start(wv_b[:], d["wv"][1024:1025, :])
            nc.gpsimd.dma_start(xt_ones[:], d["xt"][1024:1025, :])
        # xt chunks 1-3: one 3D DMA each
        for c in range(1, NCH):
            dxv = xt_all.rearrange("p (m c) -> p m c", m=MB)[
                :, :, CH * c:CH * (c + 1)]
            sxv = d["xt"][0:1024, CH * c:CH * (c + 1)].rearrange(
                "(m p) c -> p m c", m=MB)
            eng = (nc.sync, nc.scalar, nc.gpsimd)[c - 1]
            eng.dma_start(dxv, sxv)
        for p in range(NP):
            nc.gpsimd.dma_start(wo_t[p][:], d["wo"][128 * p:128 * (p + 1), :])

        # ---- filler machinery: generators each yielding ~0.5us PE units ---
        # Emission discipline: an instruction's producers must always be
        # emitted before it (engine queues are in-order; a forward
        # cross-engine dependency can deadlock).  fill() advances the FIFO
        # head; ensure(g) advances the FIFO until generator g is exhausted.
        fillers = deque()
        done = set()

        def fill(n=1):
            k = 0
            while fillers and k < n:
                try:
                    next(fillers[0])
                    k += 1
                except StopIteration:
                    done.add(id(fillers[0]))
                    fillers.popleft()

        def drain():
            while fillers:
                fill(1)

        def ensure(*gens):
            for g in gens:
                while id(g) not in done and fillers:
                    fill(1)

        def xsl(mb, c0, n):
            return xt_all[:, S * mb + c0:S * mb + c0 + n]

        def g_qk(p, which, c):
            dst = qt if which == 0 else kt
            wall = wq_all if which == 0 else wk_all
            ps = psX.tile([128, CH], F32, name="psqk", tag="px")
            for mb in range(MB):
                mm(ps[:], wall[:, 256 * mb + 128 * p:256 * mb + 128 * (p + 1)],
                   xsl(mb, CH * c, CH),
                   start=(mb == 0), stop=(zero_bias and mb == MB - 1))
                if mb % 2 == 1 and mb < MB - 1:
                    yield
            if not zero_bias:
                wb = wq_b if which == 0 else wk_b
                mm(ps[:], wb[:, 128 * p:128 * (p + 1)],
                   xt_ones[:, CH * c:CH * (c + 1)], start=False, stop=True)
            nc.vector.tensor_copy(dst[p][:, CH * c:CH * (c + 1)], ps[:])
            yield

        def g_v(j0):
            # two k-blocks share one [128,512] psum bank
            ps = psX.tile([128, 512], F32, name="psv", tag="px")
            for dj in range(2):
                j = j0 + dj
                for mb in range(MB):
                    mm(ps[:, 256 * dj:256 * (dj + 1)],
                       xsl(mb, 128 * j, 128), wv_all[:, 256 * mb:256 * (mb + 1)],
                       start=(mb == 0),
                       stop=(zero_bias and mb == MB - 1))
                    if mb % 4 == 3 and not (dj == 1 and mb == MB - 1):
                        yield
                if not zero_bias:
                    mm(ps[:, 256 * dj:256 * (dj + 1)],
                       xt_ones[:, 128 * j:128 * (j + 1)], wv_b[:],
                       start=False, stop=True)
            for dj in range(2):
                j = j0 + dj
                nc.vector.tensor_copy(vt[j][:], ps[:, 256 * dj:256 * (dj + 1)])
                if j % 4:
                    bs = psX.tile([1, 256], F32, name="psb", tag="px")
                    mm(bs[:], ones_col[:], vt[j][:], start=True, stop=True)
                    nc.vector.tensor_copy(b_sb[j][:], bs[:])
            yield

        def g_sfx():
            # suffix column-sums of V: sfx[c] = sum_{j>=4c+4} colsum(V_j),
            # built high-to-low in one accumulating psum bank with
            # mid-group snapshots.
            ps = psX.tile([1, 256], F32, name="pssfx", tag="px")
            nc.vector.memset(ps[:], 0.0)
            for c in (2, 1, 0):
                for j in range(4 * c + 7, 4 * c + 3, -1):
                    mm(ps[:], ones_col[:], vt[j][:],
                       start=False, stop=False, skip_group_check=True)
                nc.vector.tensor_copy(sfx[c][:], ps[:])
                yield

        def g_outproj(ch):
            for q in range(4 * ch, 4 * ch + 4):
                for mc in range(2):
                    ops = psX.tile([128, CH], F32, name="ops", tag="px")
                    for p in range(NP):
                        mm(ops[:], z2u[p][:, 128 * q:128 * (q + 1)],
                           wo_t[p][:, CH * mc:CH * (mc + 1)],
                           start=(p == 0), stop=(p == 1))
                    osb = op_sb.tile([128, CH], BF16, name="osb", tag="osb")
                    nc.vector.tensor_copy(osb[:], ops[:])
                    eng = nc.sync if (2 * q + mc) % 2 else nc.scalar
                    eng.dma_start(
                        d["out"][128 * q:128 * (q + 1), CH * mc:CH * (mc + 1)],
                        osb[:])
                    yield

        # ---- attention ----
        zps_t = {}

        def emit_attn_pair(ch, p, weave=2):
            nj = 4 * ch + 4
            zps = psZ.tile([128, CH], F32, name=f"zps{p}", tag=f"zps{p}")
            zps_t[p] = zps
            # Two interleaved accumulation groups (per head half) in one
            # bank can't use start=True (it pends-zero the whole 2KB
            # region): zero explicitly and accumulate throughout.
            nc.vector.memset(zps[:], 0.0)
            sps_l = {}
            pt_l = {}

            def sc(j):
                r = j - 4 * ch
                w0 = 128 * r if r > 0 else 0
                sps = psS.tile([128, 2 * CH], F32, name="sps", tag="sps")
                sps_l[j] = sps
                mm(sps[:, w0:CH],
                   kt[p][0:64, 128 * j:128 * (j + 1)],
                   qt[p][0:64, CH * ch + w0:CH * (ch + 1)],
                   start=True, stop=True)
                mm(sps[:, CH + w0:2 * CH],
                   kt[p][64:128, 128 * j:128 * (j + 1)],
                   qt[p][64:128, CH * ch + w0:CH * (ch + 1)],
                   start=True, stop=True)

            def expg(j):
                r = j - 4 * ch
                w0 = 128 * r if r > 0 else 0
                sps = sps_l.pop(j)
                sps3 = sps.rearrange("p (t c) -> p t c", t=2)
                pt = pp.tile([128, 2 * CH], BF16, name="pt", tag="pt")
                pt_l[j] = pt
                pt3 = pt.rearrange("p (t c) -> p t c", t=2)
                if r >= 0:
                    strip = sps3[:, :, w0:w0 + 128]
                    dtri3 = dtri.rearrange("p (t c) -> p t c", t=2)
                    nc.vector.tensor_mul(strip, strip, dtri3)
                if w0:
                    nc.scalar.activation(pt3[:, :, w0:CH], sps3[:, :, w0:CH],
                                         EXP, scale=0.125)
                else:
                    nc.scalar.activation(pt[:], sps[:], EXP, scale=0.125)
                # PTS accumulation for the softmax denominator
                pts3 = pts[p].rearrange("p (t c) -> p t c", t=2)
                if j == 0:
                    nc.vector.tensor_copy(pts[p][:], pt[:])
                elif w0:
                    nc.vector.tensor_add(pts3[:, :, w0:CH], pts3[:, :, w0:CH],
                                         pt3[:, :, w0:CH])
                else:
                    nc.vector.tensor_add(pts[p][:], pts[p][:], pt[:])

            def av(j):
                r = j - 4 * ch
                w0 = 128 * r if r > 0 else 0
                pt = pt_l.pop(j)
                for h in range(2):
                    hh = 2 * p + h
                    mm(zps[64 * h:64 * (h + 1), w0:CH],
                       vt[j][:, 64 * hh:64 * (hh + 1)],
                       pt[:, CH * h + w0:CH * (h + 1)],
                       start=False, stop=False, skip_group_check=True)
                if w0:
                    for h in range(2):
                        hh = 2 * p + h
                        mm(zps[64 * h:64 * (h + 1), 0:w0],
                           b_sb[j][:, 64 * hh:64 * (hh + 1)],
                           ones_row[:, 0:w0], start=False, stop=False,
                           skip_group_check=True)

            # software-pipelined: scores run one block ahead of AV
            sc(0)
            for j in range(nj):
                expg(j)
                if j + 1 < nj:
                    sc(j + 1)
                av(j)
                fill(weave)

            # suffix correction (ch<3): adds sfx[ch] everywhere, closes the
            # accumulation groups.  sfx_g must be fully emitted first
            # (emission discipline; late-bound closure).
            if ch < NCH - 1:
                ensure(sfx_g)
                for h in range(2):
                    hh = 2 * p + h
                    mm(zps[64 * h:64 * (h + 1), :],
                       sfx[ch][:, 64 * hh:64 * (hh + 1)],
                       ones_row[:], start=False, stop=False,
                       skip_group_check=True)

        def tail_a(ch, p):
            # synchronous: D matmuls (free pts for the next chunk), z
            # extraction (frees the zps bank), division DVE chain
            zps = zps_t.pop(p)
            dps = psX.tile([33, CH], F32, name="dps", tag="px")
            nc.vector.memset(dps[:], 0.0)
            for h in range(2):
                mm(dps[32 * h:32 * h + 1, :], ones_col[:],
                   pts[p][:, CH * h:CH * (h + 1)], start=False, stop=False,
                   skip_group_check=True)
            nc.vector.tensor_copy(z2u[p][:, CH * ch:CH * (ch + 1)], zps[:])
            rdcb = []
            for h in range(2):
                dnr = stg.tile([1, CH], F32, name="dnr", tag="dnr")
                nc.vector.tensor_add(dnr[:], dps[32 * h:32 * h + 1, :],
                                     dk_t[0:1, CH * ch:CH * (ch + 1)])
                rdc = stg.tile([1, CH], F32, name="rdc", tag="rdc")
                nc.vector.reciprocal_approx_fast(rdc[:], dnr[:])
                rb = stg.tile([1, CH], BF16, name="rdcb", tag=f"rdcb{p}")
                nc.vector.tensor_copy(rb[:], rdc[:])
                rdcb.append(rb)
            return rdcb

        def g_tail_b(ch, p, rdcb):
            # deferred: reciprocal broadcast + z division (PE would stall
            # on the DVE chain if emitted back-to-back with tail_a)
            bc = psX.tile([128, CH], F32, name="bc", tag="px")
            mm(bc[:], e0t[:], rdcb[0][:], start=True, stop=False)
            mm(bc[:], e1t[:], rdcb[1][:], start=False, stop=True)
            nc.vector.tensor_mul(z2u[p][:, CH * ch:CH * (ch + 1)],
                                 z2u[p][:, CH * ch:CH * (ch + 1)], bc[:])
            yield

        def run(g):
            for _ in g:
                pass

        # ---- top-level schedule ----
        # chunk-0 projections for p0 plus V[0..3] emit synchronously (they
        # gate attention chunk 0); everything else becomes weave filler.
        run(g_qk(0, 0, 0))
        run(g_qk(0, 1, 0))
        run(g_v(0))
        run(g_v(2))
        qk_g = {1: [[g_qk(p, w, 1) for w in range(2)] for p in range(NP)]}
        p1_g = [g_qk(1, 0, 0), g_qk(1, 1, 0)]
        sfx_g = g_sfx()
        fillers.extend(p1_g)
        fillers.extend([g_v(j) for j in range(4, KB, 2)])
        fillers.append(sfx_g)
        fillers.extend(qk_g[1][0])
        fillers.extend(qk_g[1][1])

        for ch in range(NCH):
            if ch + 1 < NCH and ch > 0:
                qk_g[ch + 1] = [[g_qk(p, w, ch + 1) for w in range(2)]
                                for p in range(NP)]
                fillers.extend(qk_g[ch + 1][0])
                fillers.extend(qk_g[ch + 1][1])
            for p in range(NP):
                if ch == 0 and p == 1:
                    ensure(*p1_g)
                if ch > 0:
                    ensure(*qk_g[ch][p])
                emit_attn_pair(ch, p, weave=2)
                rdcb = tail_a(ch, p)
                fillers.append(g_tail_b(ch, p, rdcb))
            if ch < NCH - 1:
                fillers.append(g_outproj(ch))
        drain()
        run(g_outproj(NCH - 1))


def build_program(zero_bias=False):
    nc = bacc.Bacc("TRN2", target_bir_lowering=False, debug=False,
                   num_devices=N_CORES)
    d = {
        "xt": nc.dram_tensor("xt", [1025, S], BF16, kind="ExternalInput").ap(),
        "wq": nc.dram_tensor("wq", [1025, 256], BF16, kind="ExternalInput").ap(),
        "wk": nc.dram_tensor("wk", [1025, 256], BF16, kind="ExternalInput").ap(),
        "wv": nc.dram_tensor("wv", [1025, 256], BF16, kind="ExternalInput").ap(),
        "wo": nc.dram_tensor("wo", [256, M], BF16, kind="ExternalInput").ap(),
        "mk": nc.dram_tensor("mk", [128, 256], BF16, kind="ExternalInput").ap(),
        "e2": nc.dram_tensor("e2", [2, 128], BF16, kind="ExternalInput").ap(),
        "cst": nc.dram_tensor("cst", [128, CH], BF16, kind="ExternalInput").ap(),
        "dk": nc.dram_tensor("dk", [1, S], F32, kind="ExternalInput").ap(),
        "out": nc.dram_tensor("out", [S, M], BF16, kind="ExternalOutput").ap(),
    }
    with tile.TileContext(nc) as tc:
        _emit(tc, nc, d, zero_bias)
    nc.compile()
    return nc


_CACHE = {}


def _get_program(zero_bias=False):
    key = ("nc", zero_bias)
    if key not in _CACHE:
        _CACHE[key] = build_program(zero_bias)
    return _CACHE[key]


def _pack_qk(w4, b4):
    # w4 [4,1024,64], b4 [4,64] -> [1025, 256] (m-major, head-major cols)
    r = np.empty((1025, 256), np.float32)
    r[:1024] = w4.transpose(1, 0, 2).reshape(1024, 256)
    r[1024] = b4.reshape(256)
    return r


def prepare_in_maps(normalized_resid_pre, W_Q, b_Q, W_K, b_K, W_V, b_V, W_O,
                    b_O):
    import ml_dtypes
    bf16 = ml_dtypes.bfloat16
    x = np.asarray(normalized_resid_pre, np.float32)
    W_Q = np.asarray(W_Q, np.float32)
    b_Q = np.asarray(b_Q, np.float32)
    W_K = np.asarray(W_K, np.float32)
    b_K = np.asarray(b_K, np.float32)
    W_V = np.asarray(W_V, np.float32)
    b_V = np.asarray(b_V, np.float32)
    W_O = np.asarray(W_O, np.float32)

    tri = np.triu(np.ones((128, 128), np.float32))  # [k,q]: 1 where k <= q
    mk = np.tile(tri, (1, 2))  # both heads of a pair side by side
    e2 = np.zeros((2, 128), np.float32)
    e2[0, :64] = 1.0
    e2[1, 64:] = 1.0
    cstv = np.ones((128, CH), np.float32)
    # analytic count of skipped masked keys per query position: each
    # contributes exp(0)=1 to the softmax denominator
    dk = (1920.0 - 128.0 * (np.arange(S) // 128)).astype(np.float32)[None, :]

    xts = []
    for b in range(2):
        xt = np.empty((1025, S), np.float32)
        xt[:1024] = x[b].T
        xt[1024] = 1.0
        xts.append(xt.astype(bf16))

    in_maps = []
    for c in range(N_CORES):
        b, g = divmod(c, 4)
        hs = slice(4 * g, 4 * g + 4)
        in_maps.append({
            "xt": xts[b],
            "wq": _pack_qk(W_Q[hs], b_Q[hs]).astype(bf16),
            "wk": _pack_qk(W_K[hs], b_K[hs]).astype(bf16),
            "wv": _pack_qk(W_V[hs], b_V[hs]).astype(bf16),
            "wo": np.ascontiguousarray(W_O[hs].reshape(256, M)).astype(bf16),
            "mk": mk.astype(bf16),
            "e2": e2.astype(bf16),
            "cst": cstv.astype(bf16),
            "dk": dk,
        })
    return in_maps


def gather(results, b_O):
    out = np.zeros((2, S, M), np.float32)
    for c in range(N_CORES):
        out[c // 4] += np.asarray(results[c]["out"], dtype=np.float32)
    out += np.asarray(b_O, np.float32)[None, None, :]
    return out


def _run(in_maps, trace=False, zero_bias=False, **kw):
    nc = _get_program(zero_bias)
    return bass_utils.run_bass_kernel_spmd(
        nc, in_maps, core_ids=list(range(N_CORES)), trace=trace, **kw)


def all_zero_bias(b_Q, b_K, b_V):
    return (not np.any(np.asarray(b_Q)) and not np.any(np.asarray(b_K))
            and not np.any(np.asarray(b_V)))


def kernel(normalized_resid_pre, W_Q, b_Q, W_K, b_K, W_V, b_V, W_O, b_O):
    in_maps = prepare_in_maps(normalized_resid_pre, W_Q, b_Q, W_K, b_K, W_V,
                              b_V, W_O, b_O)
    res = _run(in_maps, zero_bias=all_zero_bias(b_Q, b_K, b_V))
    return gather(res.results, b_O)
